# revision 8
# baseline (speedup 1.0000x reference)
"""Trainium2 Bass kernel for nn_AwareDecoder segment first/last gather.

Problem: input [16, 2048, 1024] f32, number_mask [16, 2048] int64 with ids in
[0, 512]. For each segment id i in [0, 512): find first/last row-major token
position with that id, gather those rows of the flattened input, concat ->
out [512, 2048] f32.

Strategy (8 NeuronCores, segment-sharded - no collectives):
  core c owns segments [64c, 64c+64). Each core:
    - DMAs the (tiny, 256KB) id array, extracts int64 low words,
    - computes per-segment min/max token position with an fp16 eq/select/
      reduce sweep on the vector engine. Token chunks sit on partitions and
      positions are encoded chunk-LOCALLY (values <= 256, fp16-exact) so the
      four mult/reduce passes run in the DVE 2x packed mode; the global
      position is reconstructed in the tiny post-transpose stage,
    - PE-transpose + free-axis reduce for the cross-partition combine,
    - gathers its 64 first + 64 last rows (4KB each) straight from HBM with
      one hardware indirect DMA (reads only 512KB of the 128MB input),
    - writes its [64, 2048] slice of the output.
Host concatenates the 8 slices.
"""
import numpy as np

import concourse.bass as bass
import concourse.tile as tile
from concourse import bacc, mybir
from concourse import bass_utils
from concourse.masks import make_identity

P = 128            # partitions
L = 32768          # B*S tokens
H = 1024           # hidden
NSEG = 512         # segments
NCORES = 8
SEG_PER_CORE = NSEG // NCORES            # 64
TOK_PER_PART = L // P                    # 256 tokens per partition
F32 = mybir.dt.float32
F16 = mybir.dt.float16
I32 = mybir.dt.int32


def build_nc():
    nc = bacc.Bacc("TRN2", target_bir_lowering=False, debug=False)

    x = nc.dram_tensor("x", [L, H], F32, kind="ExternalInput")
    # number_mask int64 raw bytes as int32 (lo, hi) pairs; partition p covers
    # tokens [p*256, (p+1)*256).
    idpairs = nc.dram_tensor("idpairs", [P, TOK_PER_PART, 2], I32, kind="ExternalInput")
    # per-core segment ids, replicated across partitions (the only per-core input)
    segs = nc.dram_tensor("segs", [P, SEG_PER_CORE], F16, kind="ExternalInput")
    segbase_in = nc.dram_tensor("segbase", [P, 1], F32, kind="ExternalInput")

    # packed fp16 consts: [const8 (8*256) | posmin (256) | posmax (256)]
    cpack_in = nc.dram_tensor("cpack", [P, 10 * TOK_PER_PART], F16,
                              kind="ExternalInput")
    # global-position bases for the post-transpose decode:
    # rows 0..63   (min side): base[s, p] = (127 - p) * 256
    # rows 64..127 (max side): base[s, p] = p * 256
    base_in = nc.dram_tensor("base", [P, P], F32, kind="ExternalInput")
    out = nc.dram_tensor("out", [SEG_PER_CORE, 2 * H], F32, kind="ExternalOutput")

    with tile.TileContext(nc) as tc:
        with tc.tile_pool(name="sb", bufs=1) as sb, \
             tc.tile_pool(name="big", bufs=1) as big, \
             tc.tile_pool(name="ps", bufs=1, space="PSUM") as ps:

            # ---- load ids, extract low int32 words, cast to fp16 ----
            idp_t = sb.tile([P, TOK_PER_PART, 2], I32)
            nc.sync.dma_start(idp_t[:], idpairs.ap())
            ids_f = sb.tile([P, TOK_PER_PART], F16)
            nc.vector.tensor_copy(ids_f[:], idp_t[:, :, 0])

            segs_t = sb.tile([P, SEG_PER_CORE], F16)
            nc.sync.dma_start(segs_t[:], segs.ap())
            segbase_t = sb.tile([P, 1], F32)
            nc.sync.dma_start(segbase_t[:], segbase_in.ap())
            cpack = sb.tile([P, 10 * TOK_PER_PART], F16)
            nc.scalar.dma_start(cpack[:], cpack_in.ap())
            const8_t = cpack[:, 0:8 * TOK_PER_PART].rearrange(
                "p (a t) -> p a t", a=8)
            posmin = cpack[:, 8 * TOK_PER_PART:9 * TOK_PER_PART]
            posmax = cpack[:, 9 * TOK_PER_PART:10 * TOK_PER_PART]
            base_t = sb.tile([P, P], F32)
            nc.scalar.dma_start(base_t[:], base_in.ap())

            # ---- factorized seg compare: seg - base = 8*hi + lo ----
            ids_loc = sb.tile([P, TOK_PER_PART], F16)
            nc.vector.tensor_scalar(ids_loc[:], ids_f[:], segbase_t[:, 0:1], None,
                                    op0=mybir.AluOpType.subtract)
            ids_li = sb.tile([P, TOK_PER_PART], I32)
            nc.vector.tensor_copy(ids_li[:], ids_loc[:])
            hi_i = sb.tile([P, TOK_PER_PART], I32)
            nc.vector.tensor_scalar(hi_i[:], ids_li[:], 3, None,
                                    op0=mybir.AluOpType.arith_shift_right)
            lo_i = sb.tile([P, TOK_PER_PART], I32)
            nc.vector.tensor_scalar(lo_i[:], ids_li[:], 7, None,
                                    op0=mybir.AluOpType.bitwise_and)
            hi_f = sb.tile([P, TOK_PER_PART], F16)
            nc.vector.tensor_copy(hi_f[:], hi_i[:])
            lo_f = sb.tile([P, TOK_PER_PART], F16)
            nc.vector.tensor_copy(lo_f[:], lo_i[:])

            eq_hi = sb.tile([P, 8, TOK_PER_PART], F16)
            nc.vector.tensor_tensor(
                out=eq_hi[:],
                in0=hi_f[:].unsqueeze(1).broadcast_to([P, 8, TOK_PER_PART]),
                in1=const8_t, op=mybir.AluOpType.is_equal)
            eq_lo = sb.tile([P, 8, TOK_PER_PART], F16)
            nc.vector.tensor_tensor(
                out=eq_lo[:],
                in0=lo_f[:].unsqueeze(1).broadcast_to([P, 8, TOK_PER_PART]),
                in1=const8_t, op=mybir.AluOpType.is_equal)
            eqlo_min = sb.tile([P, 8, TOK_PER_PART], F16)
            nc.vector.tensor_tensor(
                out=eqlo_min[:], in0=eq_lo[:],
                in1=posmin.unsqueeze(1).broadcast_to([P, 8, TOK_PER_PART]),
                op=mybir.AluOpType.mult)
            eqlo_max = sb.tile([P, 8, TOK_PER_PART], F16)
            nc.vector.tensor_tensor(
                out=eqlo_max[:], in0=eq_lo[:],
                in1=posmax.unsqueeze(1).broadcast_to([P, 8, TOK_PER_PART]),
                op=mybir.AluOpType.mult)

            # ---- big fused candidate passes (2x) + reduces ----
            red = sb.tile([P, P], F16)  # [:, :64] min-enc, [:, 64:] max-enc
            cand = big.tile([P, 8, 8, TOK_PER_PART], F16)
            nc.vector.tensor_tensor(
                out=cand[:],
                in0=eq_hi[:].unsqueeze(2).broadcast_to([P, 8, 8, TOK_PER_PART]),
                in1=eqlo_min[:].unsqueeze(1).broadcast_to([P, 8, 8, TOK_PER_PART]),
                op=mybir.AluOpType.mult)
            # TT-max tree (2x) then small reduce: 256 -> 32 -> 1
            red_min = sb.tile([P, SEG_PER_CORE], F16)
            c3 = cand[:].rearrange("p a b t -> p (a b) t")
            lv1 = big.tile([P, SEG_PER_CORE, 128], F16, tag="lv1")
            nc.vector.tensor_tensor(out=lv1[:], in0=c3[:, :, 0:128],
                                    in1=c3[:, :, 128:256], op=mybir.AluOpType.max)
            lv2 = sb.tile([P, SEG_PER_CORE, 64], F16, tag="lv2")
            nc.vector.tensor_tensor(out=lv2[:], in0=lv1[:, :, 0:64],
                                    in1=lv1[:, :, 64:128], op=mybir.AluOpType.max)
            lv3 = sb.tile([P, SEG_PER_CORE, 32], F16, tag="lv3")
            nc.vector.tensor_tensor(out=lv3[:], in0=lv2[:, :, 0:32],
                                    in1=lv2[:, :, 32:64], op=mybir.AluOpType.max)
            nc.vector.tensor_reduce(red_min[:], lv3[:],
                                    axis=mybir.AxisListType.X,
                                    op=mybir.AluOpType.max)
            cand2 = big.tile([P, 8, 8, TOK_PER_PART], F16)
            nc.vector.tensor_tensor(
                out=cand2[:],
                in0=eq_hi[:].unsqueeze(2).broadcast_to([P, 8, 8, TOK_PER_PART]),
                in1=eqlo_max[:].unsqueeze(1).broadcast_to([P, 8, 8, TOK_PER_PART]),
                op=mybir.AluOpType.mult)
            red_max = sb.tile([P, SEG_PER_CORE], F16)
            c3b = cand2[:].rearrange("p a b t -> p (a b) t")
            lv1b = big.tile([P, SEG_PER_CORE, 128], F16, tag="lv1")
            nc.vector.tensor_tensor(out=lv1b[:], in0=c3b[:, :, 0:128],
                                    in1=c3b[:, :, 128:256], op=mybir.AluOpType.max)
            lv2b = sb.tile([P, SEG_PER_CORE, 64], F16, tag="lv2")
            nc.vector.tensor_tensor(out=lv2b[:], in0=lv1b[:, :, 0:64],
                                    in1=lv1b[:, :, 64:128], op=mybir.AluOpType.max)
            lv3b = sb.tile([P, SEG_PER_CORE, 32], F16, tag="lv3")
            nc.vector.tensor_tensor(out=lv3b[:], in0=lv2b[:, :, 0:32],
                                    in1=lv2b[:, :, 32:64], op=mybir.AluOpType.max)
            nc.vector.tensor_reduce(red_max[:], lv3b[:],
                                    axis=mybir.AxisListType.X,
                                    op=mybir.AluOpType.max)
            nc.vector.tensor_copy(red[:, 0:SEG_PER_CORE], red_min[:])
            nc.vector.tensor_copy(red[:, SEG_PER_CORE:2 * SEG_PER_CORE], red_max[:])

            # ---- cross-partition combine: transpose, decode, free-axis max ----
            ident = sb.tile([P, P], F16)
            make_identity(nc, ident[:])
            red_t = ps.tile([P, P], F16)
            nc.tensor.transpose(out=red_t[:], in_=red[:], identity=ident[:])
            # valid chunks have local enc >= 1; empty chunks are 0
            mask = sb.tile([P, P], F32)
            nc.vector.tensor_scalar(mask[:], red_t[:], 0.0, None,
                                    op0=mybir.AluOpType.is_gt)
            glob = sb.tile([P, P], F32)
            nc.vector.tensor_tensor(out=glob[:], in0=red_t[:], in1=base_t[:],
                                    op=mybir.AluOpType.add)
            nc.vector.tensor_tensor(out=glob[:], in0=glob[:], in1=mask[:],
                                    op=mybir.AluOpType.mult)
            enc = sb.tile([P, 1], F32)
            nc.vector.tensor_reduce(enc[:], glob[:],
                                    axis=mybir.AxisListType.X,
                                    op=mybir.AluOpType.max)

            # ---- decode to global indices ----
            idx_f = sb.tile([P, 1], F32)
            # min side: first = 32768 - G
            nc.vector.tensor_scalar(idx_f[0:SEG_PER_CORE, :], enc[0:SEG_PER_CORE, :],
                                    -1.0, float(L),
                                    op0=mybir.AluOpType.mult,
                                    op1=mybir.AluOpType.add)
            # max side: last = H_enc - 1
            nc.vector.tensor_scalar_add(idx_f[SEG_PER_CORE:P, :],
                                        enc[SEG_PER_CORE:P, :], -1.0)
            idx_i = sb.tile([P, 1], I32)
            nc.vector.tensor_copy(idx_i[:], idx_f[:])

            # ---- hardware gather of 128 rows (4KB each) from HBM ----
            rows = big.tile([P, H], F32)
            nc.gpsimd.indirect_dma_start(
                out=rows[:],
                out_offset=None,
                in_=x.ap(),
                in_offset=bass.IndirectOffsetOnAxis(ap=idx_i[:, 0:1], axis=0),
            )

            # ---- write output slice ----
            nc.sync.dma_start(out.ap()[:, 0:H], rows[0:SEG_PER_CORE, :])
            nc.scalar.dma_start(out.ap()[:, H:2 * H], rows[SEG_PER_CORE:P, :])

    nc.compile()
    return nc


_NC = None


def _get_nc():
    global _NC
    if _NC is None:
        _NC = build_nc()
    return _NC


def make_in_maps(input, number_mask):
    x = np.ascontiguousarray(np.asarray(input), dtype=np.float32).reshape(L, H)
    nm = np.ascontiguousarray(np.asarray(number_mask))
    if nm.dtype != np.int64:
        nm = nm.astype(np.int64)
    idpairs = nm.reshape(L).view(np.int32).reshape(P, TOK_PER_PART, 2)
    const8 = np.repeat(np.arange(8, dtype=np.float16), TOK_PER_PART)
    f = np.arange(TOK_PER_PART, dtype=np.float16)
    cpack = np.tile(np.concatenate([const8, TOK_PER_PART - f, f + 1]), (P, 1))
    pcol = np.arange(P, dtype=np.float32)
    base = np.empty((P, P), dtype=np.float32)
    base[0:SEG_PER_CORE, :] = (P - 1 - pcol) * TOK_PER_PART
    base[SEG_PER_CORE:P, :] = pcol * TOK_PER_PART
    in_maps = []
    for c in range(NCORES):
        segs = np.tile(
            np.arange(c * SEG_PER_CORE, (c + 1) * SEG_PER_CORE, dtype=np.float16),
            (P, 1))
        in_maps.append({"x": x, "idpairs": idpairs, "segs": segs,
                        "segbase": np.full((P, 1), c * SEG_PER_CORE, np.float32),
                        "cpack": cpack, "base": base})
    return in_maps


def kernel(input, number_mask, n, concat, **_):
    assert int(n) == NSEG and int(concat) == 1
    nc = _get_nc()
    in_maps = make_in_maps(input, number_mask)
    res = bass_utils.run_bass_kernel_spmd(nc, in_maps, core_ids=list(range(NCORES)))
    return np.concatenate([res.results[c]["out"] for c in range(NCORES)], axis=0)
